# revision 17
# baseline (speedup 1.0000x reference)
"""CapsuleLinear dynamic-routing kernel for TRN2 (8 NeuronCores, data-parallel over batch).

Math (reference):
    priors[n,j,i,k] = sum_l x[n,i,l] * w[j,k,l]          (never materialized)
    3 routing iterations entirely in the L=8 compressed space:
      probs = softmax_j(logits)                          logits[n,i,j], init 0
      s[n,j,l]  = sum_i probs[n,j,i] * x[n,i,l]          (PE matmul, contraction over i)
      sq[n,j]   = s^T G s  with G[j] = W[j]^T W[j]       (= ||W s||^2, precomputed G)
      v[n,j,l]  = scale(sq) * G s                        (squash scale folded into v)
      logits   += sum_l x[n,i,l] * v[n,j,l]              (PE matmul, PSUM-resident accum)
    Final iter only: u = W s, out = scale(sq) * u.

Layout: i = 9*p + q  (p = SBUF partition 0..127, q = 0..8).
Per-sample tensors are packed (n2, j) on 128 partitions (n = 2h + n2, h = sample
half), so the whole squash/v chain runs at full partition width.
logits lives in 5 PSUM tiles of one bank each (q pairs) so the exp of early q's
overlaps the (b) matmuls of later q's. Softmax normalization is folded into x
(xs = x * 1/d per i); xT = x/J in bf16 (J folded back via the Exp bias ln(J)).
Matmul operands are bf16; all PSUM accumulation is f32.
"""

import os

import numpy as np

N, I, L, J, K = 32, 1152, 8, 64, 16
NCORES = 8
NPC = N // NCORES  # samples per core = 4
P = 128
Q = I // P  # 9
ITERS = 3
EPS = 1e-9
LN_J = float(np.log(float(J)))

_cache = {}
LAST_RESULT = None


def _patch_act_tables():
    """Restrict Exp/Ln to the one table set containing both, so bacc's
    table-load pass never alternates sets (each reload costs ~2.7us)."""
    import concourse.hw_specs as hw_specs
    from concourse import mybir

    import concourse.bacc as bacc

    if getattr(hw_specs, "_capsule_patched", False):
        return
    orig = hw_specs.get_activation_tables

    def patched(arch):
        t = dict(orig(arch))
        AF = mybir.ActivationFunctionType
        both = "natural_log_exp_and_others"
        if both in t:
            for name in t:
                if name != both:
                    t[name] = t[name] - {AF.Exp, AF.Ln}
        return t

    hw_specs.get_activation_tables = patched
    bacc.get_activation_tables = patched  # bacc binds the name via from-import
    hw_specs._capsule_patched = True


def _build():
    import concourse.bacc as bacc
    import concourse.tile as tile
    from concourse import mybir
    from concourse.masks import make_identity

    _patch_act_tables()

    f32 = mybir.dt.float32
    bf16 = mybir.dt.bfloat16
    AF = mybir.ActivationFunctionType
    AX = mybir.AxisListType
    OP = mybir.AluOpType
    dlow = bool(int(os.environ.get("DLOW", "0")))
    ddt = bf16 if dlow else f32

    nc = bacc.Bacc("TRN2", target_bir_lowering=False, debug=False, num_devices=NCORES)

    x_d = nc.dram_tensor("x", (NPC, I, L), f32, kind="ExternalInput")
    w_d = nc.dram_tensor("weight", (J, K, L), f32, kind="ExternalInput")
    o_d = nc.dram_tensor("out", (NPC, J, K), f32, kind="ExternalOutput")

    with tile.TileContext(nc) as tc:
        with tc.tile_pool(name="singles", bufs=1) as singles:
            # ---- input DMAs (2-way queue split); w replicated to both halves ----
            xall = singles.tile([P, NPC, Q, L], f32)
            xsrc = x_d[:].rearrange("n (p q) l -> p n q l", p=P)
            nc.sync.dma_start(out=xall[:, 0:2], in_=xsrc[:, 0:2])
            nc.scalar.dma_start(out=xall[:, 2:4], in_=xsrc[:, 2:4])
            w2 = singles.tile([P, K, L], f32)
            nc.sync.dma_start(out=w2[0:J], in_=w_d[:])
            nc.scalar.dma_start(out=w2[J:P], in_=w_d[:])

            # ---- constants ----
            id_t = singles.tile([P, P], bf16)
            make_identity(nc, id_t)
            ones_t = singles.tile([P, P], bf16)
            nc.gpsimd.memset(ones_t, 1.0)
            # padded v (pre-transpose): partitions (n2 j), free (h, 32-pad l)
            vT_pad = singles.tile([P, 2, 32], bf16)
            nc.gpsimd.memset(vT_pad, 0.0)
            # block-diag v: sample n occupies partitions 32n..32n+8
            vblk = singles.tile([P, NPC * J], bf16)
            nc.gpsimd.memset(vblk, 0.0)
            # zero-padded xs feeding the xT transposes
            xpad = singles.tile([P, Q, NPC, 32], bf16)
            nc.gpsimd.memset(xpad, 0.0)
            eps_t = singles.tile([P, 1], f32)
            nc.gpsimd.memset(eps_t, EPS)
            lnj_t = singles.tile([P, 1], f32)
            nc.gpsimd.memset(lnj_t, LN_J)

            # warm the ACT ln/exp table set while DMAs run
            warm = singles.tile([1, 2], f32)
            nc.gpsimd.memset(warm, 1.0)
            nc.scalar.activation(warm[:, 0:1], warm[:, 0:1], AF.Ln)
            nc.scalar.activation(warm[:, 1:2], warm[:, 1:2], AF.Exp)

            # ---- xs (iter-0 probs folded: x/J); xpad copy feeds transposes ----
            xs = singles.tile([P, Q, NPC, L], bf16)
            nc.vector.tensor_scalar_mul(xs, xall[:].transpose([0, 2, 1, 3]), 1.0 / J)
            nc.gpsimd.tensor_copy(xpad[:, :, :, 0:L], xs[:])

            # ---- G[(n2 j), l, l'] = sum_k w w': products on idle GpSimd during
            # iter 0 (which uses W directly); tree-reduce on Vector after the
            # iter-0 scatter. G is only consumed from iteration 1 on.
            w2T = w2[:].transpose([0, 2, 1])  # [P, L, K] view
            gtmp = singles.tile([P, L, L, K], bf16)
            nc.gpsimd.tensor_mul(
                gtmp,
                w2T.unsqueeze(2).broadcast_to((P, L, L, K)),
                w2T.unsqueeze(1).broadcast_to((P, L, L, K)),
            )
            gt8 = singles.tile([P, L, L, 8], bf16)
            gt4 = singles.tile([P, L, L, 4], bf16)
            gt2 = singles.tile([P, L, L, 2], bf16)
            g_t = singles.tile([P, L, L], f32)

            def g_reduce():
                nc.vector.tensor_add(gt8, gtmp[:, :, :, 0:8], gtmp[:, :, :, 8:16])
                nc.vector.tensor_add(gt4, gt8[:, :, :, 0:4], gt8[:, :, :, 4:8])
                nc.vector.tensor_add(gt2, gt4[:, :, :, 0:2], gt4[:, :, :, 2:4])
                nc.vector.tensor_add(
                    g_t.unsqueeze(3), gt2[:, :, :, 0:1], gt2[:, :, :, 1:2]
                )

            # ---- iteration temporaries ----
            xT_sb = singles.tile([P, Q, P], bf16)
            e_t = singles.tile([P, Q, NPC, J], bf16)
            d_t = singles.tile([P, Q, NPC], ddt)
            r_t = singles.tile([P, Q, NPC], ddt)
            s_sb = singles.tile([P, 2, L], f32)  # partitions (n2 j), free (h, l)
            qt2 = singles.tile([P, 2, L, L], f32)
            qk2 = singles.tile([P, 2, L, K], f32)
            vpr = singles.tile([P, 2, L], f32)
            m2 = singles.tile([P, 2, L], f32)
            sq2 = singles.tile([P, 2], f32)
            ln2 = singles.tile([P, 2], f32)
            rti = singles.tile([P, 2], f32)
            sp1 = singles.tile([P, 2], f32)
            r1 = singles.tile([P, 2], f32)
            v2t = singles.tile([P, 2, L], f32)
            v3t = singles.tile([P, 2, L], f32)
            pt2 = singles.tile([P, 2, K, L], f32)
            u2 = singles.tile([P, 2, K], f32)
            uu = singles.tile([P, 2, K], f32)
            oc1 = singles.tile([P, 2, K], f32)
            oc2 = singles.tile([P, 2, K], f32)
            oc = singles.tile([P, 2, K], f32)

            def a_phase(t):
                """(a) matmuls into s2a/s2b + diag extract to s_sb."""
                s2h = [
                    s2a_pool.tile([P, 2 * L], f32, tag="s2a", name="s2a"),
                    s2b_pool.tile([P, 2 * L], f32, tag="s2b", name="s2b"),
                ]
                for h in range(2):
                    for q in range(Q):
                        lhsT = (
                            ones_t[:]
                            if t == 0
                            else e_t[:, q, 2 * h : 2 * h + 2].rearrange(
                                "p n j -> p (n j)"
                            )
                        )
                        nc.tensor.matmul(
                            s2h[h],
                            lhsT,
                            xs[:, q, 2 * h : 2 * h + 2].rearrange("p n l -> p (n l)"),
                            start=(q == 0),
                            stop=(q == Q - 1),
                        )
                    nc.scalar.copy(s_sb[0:64, h, :], s2h[h][0:64, 0:L])
                    nc.vector.tensor_copy(
                        s_sb[64:128, h, :], s2h[h][64:128, L : 2 * L]
                    )

            def v_phase(t):
                """v' = G s, sq = ||W s||^2, scale chain, v into vT_pad."""
                if t == 0:
                    # G not ready yet: v' = W^T (W s) directly
                    nc.vector.tensor_mul(
                        pt2,
                        w2[:].unsqueeze(1).broadcast_to((P, 2, K, L)),
                        s_sb[:].unsqueeze(2).broadcast_to((P, 2, K, L)),
                    )
                    nc.vector.reduce_sum(u2, pt2, axis=AX.X)
                    nc.vector.tensor_mul(uu, u2, u2)
                    nc.vector.tensor_reduce(sq2, uu, axis=AX.X, op=OP.add)
                    nc.vector.tensor_mul(
                        qk2,
                        w2T.unsqueeze(1).broadcast_to((P, 2, L, K)),
                        u2[:].unsqueeze(2).broadcast_to((P, 2, L, K)),
                    )
                    nc.vector.reduce_sum(vpr, qk2, axis=AX.X)
                else:
                    nc.vector.tensor_mul(
                        qt2,
                        g_t[:].unsqueeze(1).broadcast_to((P, 2, L, L)),
                        s_sb[:].unsqueeze(2).broadcast_to((P, 2, L, L)),
                    )
                    nc.vector.reduce_sum(vpr, qt2, axis=AX.X)
                    nc.vector.tensor_mul(m2, s_sb, vpr)
                    nc.vector.tensor_reduce(sq2, m2, axis=AX.X, op=OP.add)
                # scale = sq/((1+sq) sqrt(sq+eps)); sqrt via exp(-0.5 ln);
                # ln(J) bias folds away the 1/J baked into xT
                nc.scalar.activation(ln2, sq2, AF.Ln, bias=eps_t[:])
                nc.scalar.activation(rti, ln2, AF.Exp, scale=-0.5, bias=lnj_t[:])
                nc.gpsimd.tensor_scalar_add(sp1, sq2, 1.0)
                nc.vector.reciprocal(r1, sp1)
                nc.gpsimd.tensor_mul(
                    v2t, vpr, sq2[:].unsqueeze(2).broadcast_to((P, 2, L))
                )
                nc.gpsimd.tensor_mul(
                    v3t, v2t, r1[:].unsqueeze(2).broadcast_to((P, 2, L))
                )
                nc.gpsimd.tensor_mul(
                    vT_pad[:, :, 0:L],
                    v3t,
                    rti[:].unsqueeze(2).broadcast_to((P, 2, L)),
                )

            def vtr_phase(t):
                """PE transpose of vT_pad; scatter sample n to vblk rows 32n."""
                vtr = vtr_pool.tile([2 * 32, P], bf16, tag="vtr", name="vtr")
                nc.tensor.transpose(
                    vtr, vT_pad[:].rearrange("p h w -> p (h w)"), id_t
                )
                for n in range(NPC):
                    h, n2 = n // 2, n % 2
                    src = vtr[32 * h : 32 * h + L, 64 * n2 : 64 * n2 + 64]
                    dst = vblk[32 * n : 32 * n + L, 64 * n : 64 * n + 64]
                    if t == 0 or n % 2 == 0:
                        nc.vector.tensor_copy(dst, src)
                    else:
                        nc.scalar.copy(dst, src)

            with tc.tile_pool(name="s2a_ps", bufs=1, space="PSUM") as s2a_pool, \
                 tc.tile_pool(name="s2b_ps", bufs=1, space="PSUM") as s2b_pool, \
                 tc.tile_pool(name="vtr_ps", bufs=1, space="PSUM") as vtr_pool:

                # ================= iteration 0 =================
                with tc.tile_pool(name="xtp_ps", bufs=2, space="PSUM") as xtp_pool:
                    a_phase(0)
                    v_phase(0)
                    # xT[32n+l, q, p] = x[n, 9p+q, l]/J via 9 PE transposes,
                    # run while the v chain occupies the other engines
                    for q in range(Q):
                        xtp = xtp_pool.tile([P, P], bf16, tag="xtp", name="xtp")
                        nc.tensor.transpose(
                            xtp, xpad[:, q].rearrange("p n w -> p (n w)"), id_t
                        )
                        nc.scalar.copy(xT_sb[:, q, :], xtp)
                    vtr_phase(0)
                    g_reduce()

                # ================= iterations 1..2 =================
                # logits as 5 one-bank PSUM tiles (q pairs) so exp of early
                # pairs only waits on their own (b) matmuls
                with tc.tile_pool(name="lp_ps", bufs=1, space="PSUM") as lp_pool:
                    lp = []
                    for g in range(5):
                        nq = 2 if g < 4 else 1
                        lpt = lp_pool.tile(
                            [P, nq, NPC, J], f32, tag=f"lp{g}", name=f"lp{g}"
                        )
                        lp.append(lpt)

                    for t in range(1, ITERS):
                        # ---- (b) matmuls with prev iter's vblk ----
                        for q in range(Q):
                            g, qq = q // 2, q % 2
                            nc.tensor.matmul(
                                lp[g][:, qq].rearrange("p n j -> p (n j)"),
                                xT_sb[:, q, :],
                                vblk[:],
                                start=(t == 1 and qq == 0),
                                stop=(t == ITERS - 1 and (qq == 1 or q == Q - 1)),
                                skip_group_check=True,
                            )
                        # ---- softmax: exp per q-pair, d/recip/xs per 3-slab ----
                        for g in range(5):
                            nq = 2 if g < 4 else 1
                            nc.scalar.activation(
                                e_t[:, 2 * g : 2 * g + nq], lp[g][:], AF.Exp
                            )
                        for g in range(3):
                            sl = slice(3 * g, 3 * g + 3)
                            nc.vector.tensor_reduce(
                                d_t[:, sl], e_t[:, sl], axis=AX.X, op=OP.add
                            )
                            nc.vector.reciprocal(r_t[:, sl], d_t[:, sl])
                            nc.gpsimd.tensor_mul(
                                xs[:, sl],
                                xall[:, :, sl, :].transpose([0, 2, 1, 3]),
                                r_t[:, sl].unsqueeze(3).broadcast_to((P, 3, NPC, L)),
                            )

                        a_phase(t)

                        if t < ITERS - 1:
                            v_phase(t)
                            vtr_phase(t)
                        else:
                            # ---- final: u = W s, out = u * scale ----
                            nc.vector.tensor_mul(
                                pt2,
                                w2[:].unsqueeze(1).broadcast_to((P, 2, K, L)),
                                s_sb[:].unsqueeze(2).broadcast_to((P, 2, K, L)),
                            )
                            nc.vector.reduce_sum(u2, pt2, axis=AX.X)
                            nc.vector.tensor_mul(uu, u2, u2)
                            nc.vector.tensor_reduce(sq2, uu, axis=AX.X, op=OP.add)
                            nc.scalar.activation(ln2, sq2, AF.Ln, bias=eps_t[:])
                            nc.scalar.activation(rti, ln2, AF.Exp, scale=-0.5)
                            nc.gpsimd.tensor_scalar_add(sp1, sq2, 1.0)
                            nc.vector.reciprocal(r1, sp1)
                            nc.vector.tensor_mul(
                                oc1, u2, sq2[:].unsqueeze(2).broadcast_to((P, 2, K))
                            )
                            nc.vector.tensor_mul(
                                oc2, oc1, r1[:].unsqueeze(2).broadcast_to((P, 2, K))
                            )
                            nc.vector.tensor_mul(
                                oc, oc2, rti[:].unsqueeze(2).broadcast_to((P, 2, K))
                            )
                            # oc[(n2 j), h, k] -> out[n, j, k], n = 2h + n2
                            nc.sync.dma_start(
                                out=o_d[:].rearrange(
                                    "(h n2) j k -> (n2 j) h k", h=2
                                ),
                                in_=oc,
                            )

    nc.finalize()
    return nc


def kernel(x, weight):
    global LAST_RESULT
    from concourse.bass_utils import run_bass_kernel_spmd

    if "nc" not in _cache:
        _cache["nc"] = _build()
    nc = _cache["nc"]

    x = np.ascontiguousarray(np.asarray(x, dtype=np.float32))
    weight = np.ascontiguousarray(np.asarray(weight, dtype=np.float32))

    in_maps = [
        {"x": x[c * NPC : (c + 1) * NPC], "weight": weight} for c in range(NCORES)
    ]
    last_exc = None
    for attempt in range(3):
        try:
            res = run_bass_kernel_spmd(nc, in_maps, core_ids=list(range(NCORES)))
            break
        except Exception as e:
            last_exc = e
            import time

            time.sleep(5 * (attempt + 1))
    else:
        raise last_exc
    LAST_RESULT = res
    return np.concatenate([r["out"] for r in res.results], axis=0)


# revision 21
# speedup vs baseline: 1.0498x; 1.0498x over previous
"""CapsuleLinear dynamic-routing kernel for TRN2 (8 NeuronCores, data-parallel over batch).

Math (reference):
    priors[n,j,i,k] = sum_l x[n,i,l] * w[j,k,l]          (never materialized)
    3 routing iterations entirely in the L=8 compressed space:
      probs = softmax_j(logits)                          logits[n,i,j], init 0
      s[n,j,l]  = sum_i probs[n,j,i] * x[n,i,l]          (PE matmul, contraction over i)
      sq[n,j]   = s^T G s  with G[j] = W[j]^T W[j]       (= ||W s||^2, precomputed G)
      v[n,j,l]  = scale(sq) * G s                        (squash scale folded into v)
      logits   += sum_l x[n,i,l] * v[n,j,l]              (PE matmul, PSUM-resident accum)
    Final iter only: u = W s, out = scale(sq) * u.

Layout: i = 9*p + q  (p = SBUF partition 0..127, q = 0..8).
Per-sample tensors are packed (n2, j) on 128 partitions (n = 2h + n2, h = sample
half), so the whole squash/v chain runs at full partition width.
logits lives in 5 PSUM tiles of one bank each (q pairs) so the exp of early q's
overlaps the (b) matmuls of later q's. Softmax normalization is folded into x
(xs = x * 1/d per i); xT = x/J in bf16 (J folded back via the Exp bias ln(J)).
Matmul operands are bf16; all PSUM accumulation is f32.
"""

import os

import numpy as np

N, I, L, J, K = 32, 1152, 8, 64, 16
NCORES = 8
NPC = N // NCORES  # samples per core = 4
P = 128
Q = I // P  # 9
ITERS = 3
EPS = 1e-9
LN_J = float(np.log(float(J)))

_cache = {}
LAST_RESULT = None


def _patch_act_tables():
    """Restrict Exp/Ln to the one table set containing both, so bacc's
    table-load pass never alternates sets (each reload costs ~2.7us)."""
    import concourse.hw_specs as hw_specs
    from concourse import mybir

    import concourse.bacc as bacc

    if getattr(hw_specs, "_capsule_patched", False):
        return
    orig = hw_specs.get_activation_tables

    def patched(arch):
        t = dict(orig(arch))
        AF = mybir.ActivationFunctionType
        both = "natural_log_exp_and_others"
        if both in t:
            for name in t:
                if name != both:
                    t[name] = t[name] - {AF.Exp, AF.Ln}
        return t

    hw_specs.get_activation_tables = patched
    bacc.get_activation_tables = patched  # bacc binds the name via from-import
    hw_specs._capsule_patched = True


def _build():
    import concourse.bacc as bacc
    import concourse.tile as tile
    from concourse import mybir
    from concourse.masks import make_identity

    _patch_act_tables()

    f32 = mybir.dt.float32
    bf16 = mybir.dt.bfloat16
    AF = mybir.ActivationFunctionType
    AX = mybir.AxisListType
    OP = mybir.AluOpType
    dlow = bool(int(os.environ.get("DLOW", "0")))
    ddt = bf16 if dlow else f32

    nc = bacc.Bacc("TRN2", target_bir_lowering=False, debug=False, num_devices=NCORES)

    x_d = nc.dram_tensor("x", (NPC, I, L), f32, kind="ExternalInput")
    w_d = nc.dram_tensor("weight", (J, K, L), f32, kind="ExternalInput")
    o_d = nc.dram_tensor("out", (NPC, J, K), f32, kind="ExternalOutput")

    with tile.TileContext(nc) as tc:
        with tc.tile_pool(name="singles", bufs=1) as singles:
            # ---- input DMAs (2-way queue split); w replicated to both halves ----
            xall = singles.tile([P, NPC, Q, L], f32)
            xsrc = x_d[:].rearrange("n (p q) l -> p n q l", p=P)
            nc.sync.dma_start(out=xall[:, 0:2], in_=xsrc[:, 0:2])
            nc.scalar.dma_start(out=xall[:, 2:4], in_=xsrc[:, 2:4])
            w2 = singles.tile([P, K, L], f32)
            nc.sync.dma_start(out=w2[0:J], in_=w_d[:])
            nc.scalar.dma_start(out=w2[J:P], in_=w_d[:])

            # ---- constants ----
            id_t = singles.tile([P, P], bf16)
            make_identity(nc, id_t)
            ones_t = singles.tile([P, P], bf16)
            nc.gpsimd.memset(ones_t, 1.0)
            # padded v (pre-transpose): partitions (n2 j), free (h, 32-pad l)
            vT_pad = singles.tile([P, 2, 32], bf16)
            nc.gpsimd.memset(vT_pad, 0.0)
            # block-diag v: sample n occupies partitions 32n..32n+8
            vblk = singles.tile([P, NPC * J], bf16)
            nc.gpsimd.memset(vblk, 0.0)
            # zero-padded xs feeding the xT transposes
            xpad = singles.tile([P, Q, NPC, 32], bf16)
            nc.gpsimd.memset(xpad, 0.0)
            eps_t = singles.tile([P, 1], f32)
            nc.gpsimd.memset(eps_t, EPS)
            lnj_t = singles.tile([P, 1], f32)
            nc.gpsimd.memset(lnj_t, LN_J)

            # warm the ACT ln/exp table set while DMAs run
            warm = singles.tile([1, 2], f32)
            nc.gpsimd.memset(warm, 1.0)
            nc.scalar.activation(warm[:, 0:1], warm[:, 0:1], AF.Ln)
            nc.scalar.activation(warm[:, 1:2], warm[:, 1:2], AF.Exp)

            # ---- xs (iter-0 probs folded: x/J); xpad copy feeds transposes ----
            xs = singles.tile([P, Q, NPC, L], bf16)
            nc.vector.tensor_scalar_mul(xs, xall[:].transpose([0, 2, 1, 3]), 1.0 / J)
            nc.gpsimd.tensor_copy(xpad[:, :, :, 0:L], xs[:])

            w2T = w2[:].transpose([0, 2, 1])  # [P, L, K] view

            # ---- iteration temporaries ----
            xT_sb = singles.tile([P, Q, P], bf16)
            e_t = singles.tile([P, Q, NPC, J], bf16)
            d_t = singles.tile([P, Q, NPC], ddt)
            r_t = singles.tile([P, Q, NPC], ddt)
            s_sb = singles.tile([P, 2, L], f32)  # partitions (n2 j), free (h, l)
            qk2 = singles.tile([P, 2, L, K], f32)
            vpr = singles.tile([P, 2, L], f32)
            sq2 = singles.tile([P, 2], f32)
            ln2 = singles.tile([P, 2], f32)
            rti = singles.tile([P, 2], f32)
            sp1 = singles.tile([P, 2], f32)
            r1 = singles.tile([P, 2], f32)
            v2t = singles.tile([P, 2, L], f32)
            v3t = singles.tile([P, 2, L], f32)
            pt2 = singles.tile([P, 2, K, L], f32)
            u2 = singles.tile([P, 2, K], f32)
            uu = singles.tile([P, 2, K], f32)
            oc1 = singles.tile([P, 2, K], f32)
            oc2 = singles.tile([P, 2, K], f32)
            oc = singles.tile([P, 2, K], f32)

            def a_phase(t):
                """(a) matmuls into s2a/s2b + diag extract to s_sb."""
                s2h = [
                    s2a_pool.tile([P, 2 * L], f32, tag="s2a", name="s2a"),
                    s2b_pool.tile([P, 2 * L], f32, tag="s2b", name="s2b"),
                ]
                for h in range(2):
                    for q in range(Q):
                        lhsT = (
                            ones_t[:]
                            if t == 0
                            else e_t[:, q, 2 * h : 2 * h + 2].rearrange(
                                "p n j -> p (n j)"
                            )
                        )
                        nc.tensor.matmul(
                            s2h[h],
                            lhsT,
                            xs[:, q, 2 * h : 2 * h + 2].rearrange("p n l -> p (n l)"),
                            start=(q == 0),
                            stop=(q == Q - 1),
                        )
                    nc.scalar.copy(s_sb[0:64, h, :], s2h[h][0:64, 0:L])
                    nc.vector.tensor_copy(
                        s_sb[64:128, h, :], s2h[h][64:128, L : 2 * L]
                    )

            def v_phase(t):
                """u = W s, sq = ||u||^2, v' = W^T u, scale chain into vT_pad."""
                nc.vector.tensor_mul(
                    pt2,
                    w2[:].unsqueeze(1).broadcast_to((P, 2, K, L)),
                    s_sb[:].unsqueeze(2).broadcast_to((P, 2, K, L)),
                )
                nc.vector.reduce_sum(u2, pt2, axis=AX.X)
                nc.vector.tensor_mul(uu, u2, u2)
                nc.vector.tensor_reduce(sq2, uu, axis=AX.X, op=OP.add)
                nc.vector.tensor_mul(
                    qk2,
                    w2T.unsqueeze(1).broadcast_to((P, 2, L, K)),
                    u2[:].unsqueeze(2).broadcast_to((P, 2, L, K)),
                )
                nc.vector.reduce_sum(vpr, qk2, axis=AX.X)
                # scale = sq/((1+sq) sqrt(sq+eps)); sqrt via exp(-0.5 ln);
                # ln(J) bias folds away the 1/J baked into xT
                nc.scalar.activation(ln2, sq2, AF.Ln, bias=eps_t[:])
                nc.scalar.activation(rti, ln2, AF.Exp, scale=-0.5, bias=lnj_t[:])
                nc.gpsimd.tensor_scalar_add(sp1, sq2, 1.0)
                nc.vector.reciprocal(r1, sp1)
                nc.gpsimd.tensor_mul(
                    v2t, vpr, sq2[:].unsqueeze(2).broadcast_to((P, 2, L))
                )
                nc.gpsimd.tensor_mul(
                    v3t, v2t, r1[:].unsqueeze(2).broadcast_to((P, 2, L))
                )
                nc.gpsimd.tensor_mul(
                    vT_pad[:, :, 0:L],
                    v3t,
                    rti[:].unsqueeze(2).broadcast_to((P, 2, L)),
                )

            def vtr_phase(t):
                """PE transpose of vT_pad; scatter sample n to vblk rows 32n."""
                vtr = vtr_pool.tile([2 * 32, P], bf16, tag="vtr", name="vtr")
                nc.tensor.transpose(
                    vtr, vT_pad[:].rearrange("p h w -> p (h w)"), id_t
                )
                for n in range(NPC):
                    h, n2 = n // 2, n % 2
                    src = vtr[32 * h : 32 * h + L, 64 * n2 : 64 * n2 + 64]
                    dst = vblk[32 * n : 32 * n + L, 64 * n : 64 * n + 64]
                    if t == 0 or n % 2 == 0:
                        nc.vector.tensor_copy(dst, src)
                    else:
                        nc.scalar.copy(dst, src)

            with tc.tile_pool(name="s2a_ps", bufs=1, space="PSUM") as s2a_pool, \
                 tc.tile_pool(name="s2b_ps", bufs=1, space="PSUM") as s2b_pool, \
                 tc.tile_pool(name="vtr_ps", bufs=1, space="PSUM") as vtr_pool:

                # ================= iteration 0 =================
                with tc.tile_pool(name="xtp_ps", bufs=2, space="PSUM") as xtp_pool:
                    a_phase(0)
                    v_phase(0)
                    # xT[32n+l, q, p] = x[n, 9p+q, l]/J via 9 PE transposes,
                    # run while the v chain occupies the other engines
                    for q in range(Q):
                        xtp = xtp_pool.tile([P, P], bf16, tag="xtp", name="xtp")
                        nc.tensor.transpose(
                            xtp, xpad[:, q].rearrange("p n w -> p (n w)"), id_t
                        )
                        nc.scalar.copy(xT_sb[:, q, :], xtp)
                    vtr_phase(0)

                # ================= iterations 1..2 =================
                # logits as 5 one-bank PSUM tiles (q pairs) so exp of early
                # pairs only waits on their own (b) matmuls
                with tc.tile_pool(name="lp_ps", bufs=1, space="PSUM") as lp_pool:
                    lp = []
                    for g in range(5):
                        nq = 2 if g < 4 else 1
                        lpt = lp_pool.tile(
                            [P, nq, NPC, J], f32, tag=f"lp{g}", name=f"lp{g}"
                        )
                        lp.append(lpt)

                    for t in range(1, ITERS):
                        # ---- (b) matmuls with prev iter's vblk ----
                        for q in range(Q):
                            g, qq = q // 2, q % 2
                            nc.tensor.matmul(
                                lp[g][:, qq].rearrange("p n j -> p (n j)"),
                                xT_sb[:, q, :],
                                vblk[:],
                                start=(t == 1 and qq == 0),
                                stop=(t == ITERS - 1 and (qq == 1 or q == Q - 1)),
                                skip_group_check=True,
                            )
                        # ---- softmax: exp per q-pair, d/recip/xs per 3-slab ----
                        for g in range(5):
                            nq = 2 if g < 4 else 1
                            nc.scalar.activation(
                                e_t[:, 2 * g : 2 * g + nq], lp[g][:], AF.Exp
                            )
                        for g in range(3):
                            sl = slice(3 * g, 3 * g + 3)
                            nc.vector.tensor_reduce(
                                d_t[:, sl], e_t[:, sl], axis=AX.X, op=OP.add
                            )
                            nc.vector.reciprocal(r_t[:, sl], d_t[:, sl])
                            nc.gpsimd.tensor_mul(
                                xs[:, sl],
                                xall[:, :, sl, :].transpose([0, 2, 1, 3]),
                                r_t[:, sl].unsqueeze(3).broadcast_to((P, 3, NPC, L)),
                            )

                        a_phase(t)

                        if t < ITERS - 1:
                            v_phase(t)
                            vtr_phase(t)
                        else:
                            # ---- final: u = W s, out = u * scale ----
                            nc.vector.tensor_mul(
                                pt2,
                                w2[:].unsqueeze(1).broadcast_to((P, 2, K, L)),
                                s_sb[:].unsqueeze(2).broadcast_to((P, 2, K, L)),
                            )
                            nc.vector.reduce_sum(u2, pt2, axis=AX.X)
                            nc.vector.tensor_mul(uu, u2, u2)
                            nc.vector.tensor_reduce(sq2, uu, axis=AX.X, op=OP.add)
                            nc.scalar.activation(ln2, sq2, AF.Ln, bias=eps_t[:])
                            nc.scalar.activation(rti, ln2, AF.Exp, scale=-0.5)
                            nc.gpsimd.tensor_scalar_add(sp1, sq2, 1.0)
                            nc.vector.reciprocal(r1, sp1)
                            nc.vector.tensor_mul(
                                oc1, u2, sq2[:].unsqueeze(2).broadcast_to((P, 2, K))
                            )
                            nc.vector.tensor_mul(
                                oc2, oc1, r1[:].unsqueeze(2).broadcast_to((P, 2, K))
                            )
                            nc.vector.tensor_mul(
                                oc, oc2, rti[:].unsqueeze(2).broadcast_to((P, 2, K))
                            )
                            # oc[(n2 j), h, k] -> out[n, j, k], n = 2h + n2
                            nc.sync.dma_start(
                                out=o_d[:].rearrange(
                                    "(h n2) j k -> (n2 j) h k", h=2
                                ),
                                in_=oc,
                            )

    nc.finalize()
    return nc


def kernel(x, weight):
    global LAST_RESULT
    from concourse.bass_utils import run_bass_kernel_spmd

    if "nc" not in _cache:
        _cache["nc"] = _build()
    nc = _cache["nc"]

    x = np.ascontiguousarray(np.asarray(x, dtype=np.float32))
    weight = np.ascontiguousarray(np.asarray(weight, dtype=np.float32))

    in_maps = [
        {"x": x[c * NPC : (c + 1) * NPC], "weight": weight} for c in range(NCORES)
    ]
    last_exc = None
    for attempt in range(3):
        try:
            res = run_bass_kernel_spmd(nc, in_maps, core_ids=list(range(NCORES)))
            break
        except Exception as e:
            last_exc = e
            import time

            time.sleep(5 * (attempt + 1))
    else:
        raise last_exc
    LAST_RESULT = res
    return np.concatenate([r["out"] for r in res.results], axis=0)
